# revision 1
# baseline (speedup 1.0000x reference)
"""Deformable Conv2d on 8 Trainium2 NeuronCores.

Sharding: core k -> (batch b = k//2, image row-half yh = k%2).
Each core handles 2048 output pixels (32 rows x 64 cols), all 9 taps,
full C=256 / F=256.

Per-core device pipeline (all bf16 compute, f32 psum accumulation):
  1. cast x[b] -> bf16, round-trip to DRAM scratch (gather source).
  2. coords/weights/indices from offsets on DVE (int-convert floor with
     round-up correction; tap grid matches the reference's meshgrid quirk).
  3. gpsimd.dma_gather per (tap, y-corner): each descriptor fetches the
     (x0, x0+1) channel-pair row (1KB) -> layout [128 samples, 512].
  4. bilinear blend as 1 tensor_scalar + 3 scalar_tensor_tensor ops with
     per-partition weights -> deform[sample, c] bf16.
  5. PE transpose deform tiles -> deformT[c, sample] (stage-2 lhsT).
  6. 18 accumulating matmuls per 128-pixel tile: out_psum[px, f] +=
     deformT[c,px].T @ W[n][c,f]; copy psum -> f32 out, DMA store.
Bias is added on host during unshard (zeros in this problem, exact add).
"""

import numpy as np

B, IH, IW, C = 4, 64, 64, 256
KH, KW, F = 3, 3, 256
N = KH * KW
HALF = IH // 2           # 32 rows per core
PX = HALF * IW           # 2048 pixels per core
NJ = PX // 128           # 16 column-tiles of 128 pixels
NCORES = 8

_cache = {}


def _host_consts(yh):
    # base grid planes [128, N, NJ]: col (n, j), partition p, pixel = j*128+p
    # Tap grid offsets reproduce the reference's meshgrid-stack-reshape quirk:
    # init = stack(meshgrid(0..2, 0..2, ij)).reshape(-1, 2), which interleaves
    # the row/col planes instead of pairing (n//3, n%3).
    flat = np.array([0, 0, 0, 1, 1, 1, 2, 2, 2, 0, 1, 2, 0, 1, 2, 0, 1, 2])
    DY = flat[0::2]
    DX = flat[1::2]
    p = np.arange(128)
    j = np.arange(NJ)
    px = j[None, :] * 128 + p[:, None]          # [128, NJ] local pixel id
    Y = yh * HALF + px // IW                    # global row
    X = px % IW
    baseY = (Y[:, None, :] - 1 + DY[None, :, None]).astype(np.float32)
    baseX = (X[:, None, :] - 1 + DX[None, :, None]).astype(np.float32)
    return baseY.reshape(128, N * NJ), baseX.reshape(128, N * NJ)


def _build_bass():
    import os
    ABL = os.environ.get("BASS_ABLATE", "")
    import concourse.bass as bass
    import concourse.mybir as mybir
    import concourse.tile as tile
    from concourse import bacc

    from concourse import library_config

    dt = mybir.dt
    Alu = mybir.AluOpType
    # dma_gather descriptor pairs must fit the SWDGE ring (size//16 entries,
    # carved out of SBUF per partition): 32KB -> 2048 entries; gathers are
    # split into 1024-index calls so two stay in flight.
    nc = bacc.Bacc(None, target_bir_lowering=False,
                   dynamic_dma_scratch_size=32768)

    xin = nc.dram_tensor("x", [IH * IW, C], dt.float32, kind="ExternalInput")
    offs_in = nc.dram_tensor("offs", [PX, 2 * N], dt.float32, kind="ExternalInput")
    w_in = nc.dram_tensor("w", [N, C, F], dt.float32, kind="ExternalInput")
    baseY_in = nc.dram_tensor("baseY", [128, N * NJ], dt.float32, kind="ExternalInput")
    baseX_in = nc.dram_tensor("baseX", [128, N * NJ], dt.float32, kind="ExternalInput")
    ident_in = nc.dram_tensor("ident", [128, 128], dt.bfloat16, kind="ExternalInput")
    out_t = nc.dram_tensor("out", [PX, F], dt.float32, kind="ExternalOutput")
    import os as _os
    DBG = bool(int(_os.environ.get("BASS_DEBUG_DUMP", "0")))
    if DBG:
        dbg_g = nc.dram_tensor("dbg_g", [128, 2, 8 * 2 * C], dt.bfloat16, kind="ExternalOutput")
        dbg_dfm = nc.dram_tensor("dbg_dfm", [128, 8 * C], dt.bfloat16, kind="ExternalOutput")
        dbg_dT = nc.dram_tensor("dbg_dT", [128, 2 * 128], dt.bfloat16, kind="ExternalOutput")
        dbg_w = nc.dram_tensor("dbg_w", [128, 4 * N * NJ], dt.float32, kind="ExternalOutput")
        dbg_ix = nc.dram_tensor("dbg_ix", [128, 2 * N * 128], dt.int16, kind="ExternalOutput")

    NPLANE = N * NJ  # 144

    with tile.TileContext(nc) as tc:
        with tc.tile_pool(name="dram", bufs=1, space="DRAM") as dpool:
            xbf_dram = dpool.tile([IH * IW + 1, C], dt.bfloat16)
            idx_dram = dpool.tile([128 * 2 * NPLANE], dt.int16)

            with tc.tile_pool(name="main", bufs=1) as pool:
                nc.gpsimd.load_library(library_config.attnmlp)
                # Warm the Q7 library IRAM (~6us load on first custom inst)
                # during the x-cast window: a minimal gather from the f32
                # input, result unused.
                warm_idx = pool.tile([128, 8], dt.int16)
                warm_out = pool.tile([128, 1, 64], dt.float32)
                nc.vector.memset(warm_idx[:], 0)
                nc.gpsimd.dma_gather(
                    out_ap=warm_out[:],
                    in_ap=bass.AP(xin, 0, [[64, 128], [1, 64]]),
                    idxs_ap=warm_idx[:],
                    num_idxs=128,
                    num_idxs_reg=128,
                    elem_size=64,
                    elem_step=64,
                )
                # ---- constants / weights ----
                wb = pool.tile([128, N, 2, F], dt.bfloat16)     # Wb[c%128, n, ch, f]
                nc.gpsimd.dma_start(
                    wb[:],
                    bass.AP(w_in, 0, [[F, 128], [128 * F, 2 * N], [1, F]]),
                )
                ident = pool.tile([128, 128], dt.bfloat16)
                nc.sync.dma_start(ident[:], ident_in[:])
                baseY = pool.tile([128, NPLANE], dt.float32)
                baseX = pool.tile([128, NPLANE], dt.float32)
                nc.scalar.dma_start(baseY[:], baseY_in[:])
                nc.scalar.dma_start(baseX[:], baseX_in[:])
                # offsets: [128, j, 18] (partition = px%128)
                offs = pool.tile([128, NJ, 2 * N], dt.float32)
                nc.scalar.dma_start(
                    offs[:],
                    bass.AP(offs_in, 0, [[2 * N, 128], [128 * 2 * N, NJ], [1, 2 * N]]),
                )

                # ---- x -> bf16 DRAM scratch ----
                with tc.tile_pool(name="xstage", bufs=1) as xpool:
                    # single DRAM->DRAM cast dma (SWDGE does the f32->bf16)
                    nc.gpsimd.dma_start(
                        bass.AP(xbf_dram.tensor, 0, [[C, 128],
                                [128 * C, IH * IW // 128], [1, C]]),
                        bass.AP(xin, 0, [[C, 128],
                                [128 * C, IH * IW // 128], [1, C]]),
                    )
                    zrow = xpool.tile([1, C], dt.bfloat16)
                    nc.vector.memset(zrow[:], 0.0)
                    nc.sync.dma_start(
                        bass.AP(xbf_dram.tensor, IH * IW * C, [[C, 1], [1, C]]),
                        zrow[:],
                    )

                # ---- coordinates / weights / indices (DVE, f32) ----
                def offview(d):
                    # [128, (n, j)] view of offs: element (p, n, j) at offs[p, j, 2n+d]
                    return bass.AP(offs.tensor, offs[:].offset + d,
                                   [[offs[:].ap[0][0], 128], [2, N], [2 * N, NJ]])

                cy = pool.tile([128, NPLANE], dt.float32)
                cx = pool.tile([128, NPLANE], dt.float32)
                fy = pool.tile([128, NPLANE], dt.float32)
                fx = pool.tile([128, NPLANE], dt.float32)
                y0 = pool.tile([128, NPLANE], dt.float32)
                x0 = pool.tile([128, NPLANE], dt.float32)
                y1 = pool.tile([128, NPLANE], dt.float32)
                uy = pool.tile([128, NPLANE], dt.float32)
                vx = pool.tile([128, NPLANE], dt.float32)
                w00 = pool.tile([128, NPLANE], dt.float32)
                w01 = pool.tile([128, NPLANE], dt.float32)
                w10 = pool.tile([128, NPLANE], dt.float32)
                w11 = pool.tile([128, NPLANE], dt.float32)
                idxc = pool.tile([128, 2 * NPLANE], dt.int16)
                idf = pool.tile([128, NPLANE], dt.float32)

                itmp = pool.tile([128, NPLANE], dt.int32)
                neg = pool.tile([128, NPLANE], dt.float32)

                def floor_into(dst_i, dst_f, src):
                    # dst_i = int(src) (trunc or round-nearest, HW-dependent);
                    # dst_f = frac; fix up if conversion rounded up.
                    nc.vector.tensor_copy(itmp[:], src)
                    nc.vector.tensor_copy(dst_i[:], itmp[:])
                    nc.vector.tensor_tensor(dst_f[:], src, dst_i[:], Alu.subtract)
                    nc.vector.tensor_scalar(neg[:], dst_f[:], 0.0, None, Alu.is_lt)
                    nc.vector.tensor_tensor(dst_i[:], dst_i[:], neg[:], Alu.subtract)
                    nc.vector.tensor_tensor(dst_f[:], dst_f[:], neg[:], Alu.add)

                nc.vector.tensor_tensor(cy[:], baseY[:], offview(0), Alu.add)
                nc.vector.tensor_scalar(cy[:], cy[:], 0.0, float(IH - 1), Alu.max, Alu.min)
                nc.vector.tensor_tensor(cx[:], baseX[:], offview(1), Alu.add)
                nc.vector.tensor_scalar(cx[:], cx[:], 0.0, float(IW - 1), Alu.max, Alu.min)
                floor_into(y0, fy, cy[:])
                floor_into(x0, fx, cx[:])
                nc.vector.tensor_scalar(y1[:], y0[:], 1.0, float(IH - 1), Alu.add, Alu.min)
                nc.vector.tensor_scalar(uy[:], fy[:], -1.0, 1.0, Alu.mult, Alu.add)
                nc.vector.tensor_scalar(vx[:], fx[:], -1.0, 1.0, Alu.mult, Alu.add)
                nc.vector.tensor_tensor(w00[:], uy[:], vx[:], Alu.mult)
                nc.vector.tensor_tensor(w01[:], uy[:], fx[:], Alu.mult)
                nc.vector.tensor_tensor(w10[:], fy[:], vx[:], Alu.mult)
                nc.vector.tensor_tensor(w11[:], fy[:], fx[:], Alu.mult)
                # idx = y*64 + x0 (exact in f32), cast to int16.
                # idxc col order: (n, yc, j) -> col = n*32 + yc*16 + j
                def idxc_view(yc):
                    return bass.AP(idxc.tensor, idxc[:].offset + yc * NJ,
                                   [[idxc[:].ap[0][0], 128], [2 * NJ, N], [1, NJ]])

                nc.vector.scalar_tensor_tensor(idf[:], y0[:], float(IW), x0[:],
                                               Alu.mult, Alu.add)
                nc.vector.tensor_copy(idxc_view(0), idf[:])
                nc.vector.scalar_tensor_tensor(idf[:], y1[:], float(IW), x0[:],
                                               Alu.mult, Alu.add)
                nc.vector.tensor_copy(idxc_view(1), idf[:])

                # ---- idx rearrange to wrapped [16, num/16] layout, replicated ----
                # target idxw[q, (n*2+yc)*128 + j*8 + a] = idxc[16a+q, n*32+yc*16+j]
                # step 1: 8 DMAs (per a) SBUF -> DRAM wrapped layout
                NW = 2 * N * 128  # 2304 cols of the wrapped plane
                for a in range(8):
                    nc.scalar.dma_start(
                        bass.AP(idx_dram.tensor, a,
                                [[NW, 16], [128, 2 * N], [8, NJ]]),
                        bass.AP(idxc.tensor,
                                idxc[:].offset + 16 * a * idxc[:].ap[0][0],
                                [[idxc[:].ap[0][0], 16], [NJ, 2 * N], [1, NJ]]),
                    )
                # step 2: 8 DMAs (per k) DRAM -> SBUF, replicating to all 128 parts
                idxw = pool.tile([128, NW], dt.int16)
                for k in range(8):
                    nc.scalar.dma_start(
                        bass.AP(idxw.tensor,
                                idxw[:].offset + 16 * k * idxw[:].ap[0][0],
                                [[idxw[:].ap[0][0], 16], [1, NW]]),
                        bass.AP(idx_dram.tensor, 0, [[NW, 16], [1, NW]]),
                    )

                # ---- main per-tap pipeline ----
                deformT = pool.tile([128, 2, N, NJ, 128], dt.bfloat16)
                xview = bass.AP(xbf_dram.tensor, 0, [[C, IH * IW], [1, 2 * C]])

                with (
                    tc.tile_pool(name="gpool", bufs=3) as gpool,
                    tc.tile_pool(name="dpool2", bufs=3) as dfpool,
                    tc.tile_pool(name="pspool", bufs=6, space="PSUM") as pspool,
                    tc.tile_pool(name="opsum", bufs=2, space="PSUM") as opsum,
                    tc.tile_pool(name="ost", bufs=2) as opool,
                ):
                    JH = NJ // 2  # 8 j-tiles per gather call (1024 indices)
                    for jh in range(2):
                        for n in range(N):
                            gy = []
                            for yc in ([] if "gather" in ABL else range(2)):
                                g = gpool.tile([128, JH, 2 * C], dt.bfloat16,
                                               tag=f"g{yc}")
                                base = (n * 2 + yc) * 128 + jh * 64
                                nc.gpsimd.dma_gather(
                                    out_ap=g[:],
                                    in_ap=xview,
                                    idxs_ap=idxw[:, base:base + 64],
                                    num_idxs=JH * 128,
                                    num_idxs_reg=JH * 128,
                                    elem_size=2 * C,
                                    elem_step=C,
                                )
                                gy.append(g)
                            if DBG and n == 4 and jh == 0:
                                nc.sync.dma_start(
                                    bass.AP(dbg_g, yc * 8 * 2 * C,
                                            [[2 * 8 * 2 * C, 128], [1, 8 * 2 * C]]),
                                    g[:])
                            dfm = dfpool.tile([128, JH, C], dt.bfloat16, tag="dfm")
                            for jl in ([] if "blend" in ABL else range(JH)):
                                j = jh * JH + jl
                                col = n * NJ + j
                                dv = dfm[:, jl, :]
                                # op1 on ACT (activation-copy with per-partition
                                # scale); fused MACs on DVE, every 4th tile's
                                # on GPSIMD to balance engines.
                                nc.scalar.activation(
                                    dv, gy[0][:, jl, 0:C],
                                    mybir.ActivationFunctionType.Copy,
                                    scale=w00[:, col:col + 1])
                                eng = nc.vector  # walrus: TensorScalarPtr not valid on Pool
                                eng.scalar_tensor_tensor(
                                    dv, gy[0][:, jl, C:2 * C], w01[:, col:col + 1], dv,
                                    Alu.mult, Alu.add)
                                eng.scalar_tensor_tensor(
                                    dv, gy[1][:, jl, 0:C], w10[:, col:col + 1], dv,
                                    Alu.mult, Alu.add)
                                eng.scalar_tensor_tensor(
                                    dv, gy[1][:, jl, C:2 * C], w11[:, col:col + 1], dv,
                                    Alu.mult, Alu.add)
                            if DBG and n == 4 and jh == 0:
                                nc.sync.dma_start(
                                    bass.AP(dbg_dfm, 0, [[8 * C, 128], [1, 8 * C]]),
                                    dfm[:])
                            for jl in ([] if "tpose" in ABL else range(JH)):
                                j = jh * JH + jl
                                for ch in range(2):
                                    pst = pspool.tile([128, 128], dt.bfloat16,
                                                      tag="pst")
                                    nc.tensor.transpose(
                                        pst[:], dfm[:, jl, ch * 128:(ch + 1) * 128],
                                        ident[:])
                                    nc.scalar.copy(deformT[:, ch, n, j, :], pst[:])

                        # ---- stage 2 for this j-half (overlaps next half) ----
                        for j in ([] if "mm" in ABL else
                                  range(jh * JH, (jh + 1) * JH)):
                            pso = opsum.tile([128, F], dt.float32, tag="pso")
                            for n2 in range(N):
                                for ch in range(2):
                                    nc.tensor.matmul(
                                        pso[:],
                                        lhsT=deformT[:, ch, n2, j, :],
                                        rhs=wb[:, n2, ch, :],
                                        start=(n2 == 0 and ch == 0),
                                        stop=(n2 == N - 1 and ch == 1),
                                    )
                            osb = opool.tile([128, F], dt.float32, tag="osb")
                            nc.scalar.copy(osb[:], pso[:])
                            nc.sync.dma_start(
                                bass.AP(out_t, j * 128 * F, [[F, 128], [1, F]]),
                                osb[:],
                            )

                    if DBG:
                        nc.sync.dma_start(
                            bass.AP(dbg_dT, 0, [[2 * 128, 128], [1, 2 * 128]]),
                            deformT[:, :, 4, 0, :])
                        nc.sync.dma_start(
                            bass.AP(dbg_w, 0, [[4 * N * NJ, 128], [1, N * NJ]]),
                            w00[:])
                        nc.sync.dma_start(
                            bass.AP(dbg_w, N * NJ, [[4 * N * NJ, 128], [1, N * NJ]]),
                            w01[:])
                        nc.sync.dma_start(
                            bass.AP(dbg_w, 2 * N * NJ, [[4 * N * NJ, 128], [1, N * NJ]]),
                            w10[:])
                        nc.sync.dma_start(
                            bass.AP(dbg_w, 3 * N * NJ, [[4 * N * NJ, 128], [1, N * NJ]]),
                            w11[:])
                        nc.sync.dma_start(
                            bass.AP(dbg_ix, 0, [[2 * N * 128, 128], [1, 2 * N * 128]]),
                            idxw[:])
    nc.compile()
    return nc


def kernel(**inputs):
    from concourse.bass_utils import run_bass_kernel_spmd

    x = np.asarray(inputs["x"], dtype=np.float32)
    offsets = np.asarray(inputs["offsets"], dtype=np.float32)
    W = np.asarray(inputs["W"], dtype=np.float32)
    b = np.asarray(inputs["b"], dtype=np.float32)

    if "nc" not in _cache:
        _cache["nc"] = _build_bass()
    nc = _cache["nc"]

    import ml_dtypes
    ident = np.eye(128).astype(ml_dtypes.bfloat16)

    in_maps = []
    for k in range(NCORES):
        bb, yh = k // 2, k % 2
        bY, bX = _host_consts(yh)
        in_maps.append({
            "x": np.ascontiguousarray(x[bb].reshape(IH * IW, C)),
            "offs": np.ascontiguousarray(
                offsets[bb, yh * HALF:(yh + 1) * HALF].reshape(PX, 2 * N)),
            "w": np.ascontiguousarray(W),
            "baseY": bY, "baseX": bX, "ident": ident,
        })

    import os
    trace = bool(int(os.environ.get("BASS_DEFORM_TRACE", "0")))
    res = run_bass_kernel_spmd(nc, in_maps, core_ids=list(range(NCORES)),
                               trace=trace)
    _cache["last_result"] = res
    out = np.empty((B, IH, IW, F), dtype=np.float32)
    for k in range(NCORES):
        bb, yh = k // 2, k % 2
        out[bb, yh * HALF:(yh + 1) * HALF] = (
            res.results[k]["out"].reshape(HALF, IW, F))
    out += b  # bias (zeros in this problem; exact elementwise add)
    return out

